# revision 9
# baseline (speedup 1.0000x reference)
"""Masked multi-head attention (B=32, Lq=Lk=512, H=20, D=20) on 8 TRN2 NeuronCores.

Strategy:
  - Data-parallel over batch: 32 batches -> 8 cores x 4 "slots" (SPMD: one NEFF).
  - Host bakes per-slot static shapes (nq = padded Q_len, nkc = kv chunks from
    V_len) and bin-packs batches into slot groups to minimize padded work.
  - Host pre-transposes sequences to [21, L] (20 features + ones row).  The
    ones row realizes: exact linear bias, zeroing of masked kv positions
    (mask folded into V/K inputs), and a free softmax-denominator column in
    the projected V tile.
  - Device per (slot, head-group of 4 heads at 32-partition offsets):
      proj Q/K/V (PE, contraction 21)
      S^T = K_h @ Q_h^T   row-tiled 4 heads concurrently  (PSUM)
      P^T = exp(S^T / sqrt(D))  one ACTIVATE per pack     (SBUF)
      O^T(+sums row) = [V_h|mask]^T @ P^T  col-tiled, accumulated over kv chunks
      PE transpose -> [q, .] layout; DVE reciprocal + broadcast multiply
      assemble [128, 400] and DMA to DRAM.
  - Host scatters per-slot outputs into the final [32, 512, 400] (rows beyond
    Q_len stay zero, which implements the multiplicative q mask exactly).
"""

import math
import random

import numpy as np

import concourse.bacc as bacc
import concourse.bass as bass
import concourse.tile as tile
from concourse import mybir
from concourse.bass_utils import run_bass_kernel_spmd

B, LQ, LK = 32, 512, 512
H, D = 20, 20
OUT_DIM = H * D  # 400
N_CORES = 8
N_SLOTS = B // N_CORES  # 4
QCH = 128
KCH = 128
NG = 5  # head groups
HPG = 4  # heads per group (at partition offsets 0/32/64/96)
VW = H * 21 + 11  # 431: per-head 20 dims + 1 ones col, padded so a 32-wide
                  # lhsT slice exists for every head (fills whole 32-part block)
SCALE = 1.0 / math.sqrt(D)

F32 = mybir.dt.float32

# Perf knobs
USE_F32R = False  # bitcast matmul operands to float32r (fast fp32 path)
TRACE = False  # set True to capture NTFF profile (slower)
LAST_RESULT = None  # BassKernelResults of the last run (for test harness)


# ----------------------------------------------------------------- planning

def _plan(q_len, v_len):
    """Group 32 batches into N_SLOTS groups of N_CORES, minimizing baked cost.

    Returns list of (nq, nkc, batches[8]) sorted big->small."""
    nqc = [max(1, math.ceil(min(int(q), LQ) / QCH)) for q in q_len]
    kv_eff = [LK if int(v) <= 0 else min(int(v), LK) for v in v_len]
    nkc = [math.ceil(k / KCH) for k in kv_eff]
    cost = [a * b for a, b in zip(nqc, nkc)]
    order = sorted(range(B), key=lambda b: -cost[b])

    def baked(gs):
        t = 0
        for g in gs:
            if g:
                t += max(nqc[b] for b in g) * max(nkc[b] for b in g)
        return t

    groups = [[] for _ in range(N_SLOTS)]
    for b in order:
        best, bestc = None, None
        for gi in range(N_SLOTS):
            if len(groups[gi]) >= N_CORES:
                continue
            groups[gi].append(b)
            c = baked(groups)
            groups[gi].pop()
            if bestc is None or c < bestc:
                best, bestc = gi, c
        groups[best].append(b)
    rng = random.Random(0)
    cur = baked(groups)
    for _ in range(6000):
        g1, g2 = rng.randrange(N_SLOTS), rng.randrange(N_SLOTS)
        if g1 == g2:
            continue
        i1, i2 = rng.randrange(N_CORES), rng.randrange(N_CORES)
        groups[g1][i1], groups[g2][i2] = groups[g2][i2], groups[g1][i1]
        c = baked(groups)
        if c <= cur:
            cur = c
        else:
            groups[g1][i1], groups[g2][i2] = groups[g2][i2], groups[g1][i1]
    slots = []
    for g in groups:
        snq = max(nqc[b] for b in g) * QCH
        snkc = max(nkc[b] for b in g)
        slots.append((snq, snkc, list(g)))
    slots.sort(key=lambda s: -(s[0] * s[1]))
    return slots


# ------------------------------------------------------------ host packing

def _pack_qk_weights(W, bias):
    """[400, 20] linear weight -> [21, NG*128] lhsT layout (head 4g+j at
    columns 128g+32j .. +20; row 20 = bias)."""
    t = np.zeros((D + 1, NG * 128), np.float32)
    for h in range(H):
        g, j = divmod(h, HPG)
        c = g * 128 + 32 * j
        t[:D, c:c + D] = W[h * D:(h + 1) * D, :].T
        t[D, c:c + D] = bias[h * D:(h + 1) * D]
    return t


def _pack_v_weights(W, bias):
    """[400, 20] -> [21, 420] rhs layout: head h at cols 21h..21h+19,
    ones-generator col at 21h+20."""
    t = np.zeros((D + 1, VW), np.float32)
    for h in range(H):
        c = 21 * h
        t[:D, c:c + D] = W[h * D:(h + 1) * D, :].T
        t[D, c:c + D] = bias[h * D:(h + 1) * D]
        t[D, c + D] = 1.0
    return t


def _prep_qt(qs, nq):
    t = np.zeros((D + 1, nq), np.float32)
    n = min(nq, LQ)
    t[:D, :n] = qs[:n].T
    t[D, :n] = 1.0
    return t


def _prep_kvt(ks, vlen, nkv):
    """K/V sequence transposed with ones row; columns >= V_len zeroed
    (vlen==0 means "uniform -1e12 shift" in the reference == full attention)."""
    t = np.zeros((D + 1, nkv), np.float32)
    n = min(nkv, LK) if int(vlen) <= 0 else min(nkv, int(vlen))
    t[:D, :n] = ks[:n].T
    t[D, :n] = 1.0
    return t


# ------------------------------------------------------------ device build

def _mm(ap):
    return ap.bitcast(mybir.dt.float32r) if USE_F32R else ap


def _emit(tc, nc, dr, slots):
    with (
        tc.tile_pool(name="wpool", bufs=1) as wpool,
        tc.tile_pool(name="seqin", bufs=2) as seqp,
        tc.tile_pool(name="sbq", bufs=3) as sbqp,
        tc.tile_pool(name="sbk", bufs=3) as sbkp,
        tc.tile_pool(name="sbv", bufs=6) as sbvp,
        tc.tile_pool(name="sbp", bufs=4) as sbpp,
        tc.tile_pool(name="sbo", bufs=2) as sbop,
        tc.tile_pool(name="sbr", bufs=4) as sbrp,
        tc.tile_pool(name="asm", bufs=6) as asmp,
        tc.tile_pool(name="ppj", bufs=2, space="PSUM") as ppj,
        tc.tile_pool(name="pss", bufs=2, space="PSUM") as pss,
        tc.tile_pool(name="pso", bufs=1, space="PSUM") as pso,
        tc.tile_pool(name="pst", bufs=1, space="PSUM") as pst,
    ):
        wq = wpool.tile([D + 1, NG * 128], F32, tag="wq")
        nc.sync.dma_start(wq[:], dr["wq"])
        wk = wpool.tile([D + 1, NG * 128], F32, tag="wk")
        nc.sync.dma_start(wk[:], dr["wk"])
        wv = wpool.tile([D + 1, VW], F32, tag="wv")
        nc.sync.dma_start(wv[:], dr["wv"])
        ident = wpool.tile([128, 128], F32, tag="ident")
        nc.sync.dma_start(ident[:], dr["ident"])

        for s, (nq, nkc, _g) in enumerate(slots):
            nkv = nkc * KCH
            nqc = nq // QCH
            # 2 heads per S^T psum tile; each head's [128, nq] slice padded to a
            # full 2KB bank so no two matmul outputs share a PSUM zero region.
            hp = 2

            qt = seqp.tile([D + 1, nq], F32, tag="qt")
            nc.sync.dma_start(qt[:], dr[f"qt{s}"])
            kt = seqp.tile([D + 1, nkv], F32, tag="kt")
            nc.sync.dma_start(kt[:], dr[f"kt{s}"])
            vt = seqp.tile([D + 1, nkv], F32, tag="vt")
            nc.sync.dma_start(vt[:], dr[f"vt{s}"])

            # V projection: per kv chunk -> [128, 420] (incl. masked ones cols)
            sbV = []
            for kc in range(nkc):
                pv = ppj.tile([128, 512], F32, tag="ppj")
                nc.tensor.matmul(
                    pv[:, :VW], _mm(vt[:, kc * KCH:(kc + 1) * KCH]), _mm(wv[:]),
                    start=True, stop=True,
                )
                v = sbvp.tile([128, VW], F32, tag="sbv")
                nc.vector.tensor_copy(v[:], pv[:, :VW])
                sbV.append(v)

            asms = [
                asmp.tile([128, OUT_DIM], F32, tag="asm", name=f"asm{s}_{qc}")
                for qc in range(nqc)
            ]

            for g in range(NG):
                pq = ppj.tile([128, 512], F32, tag="ppj")
                nc.tensor.matmul(
                    pq[:, :nq], _mm(wq[:, g * 128:(g + 1) * 128]), _mm(qt[:]),
                    start=True, stop=True,
                )
                q = sbqp.tile([128, nq], F32, tag="sbq")
                nc.vector.tensor_copy(q[:], pq[:, :nq])

                pk = ppj.tile([128, 512], F32, tag="ppj")
                nc.tensor.matmul(
                    pk[:, :nkv], _mm(wk[:, g * 128:(g + 1) * 128]), _mm(kt[:]),
                    start=True, stop=True,
                )
                k = sbkp.tile([128, nkv], F32, tag="sbk")
                nc.vector.tensor_copy(k[:], pk[:, :nkv])

                po = pso.tile([128, nq], F32, tag="pso")

                for kc in range(nkc):
                    for jp in range(0, HPG, hp):
                        ps = pss.tile([128, hp, 512], F32, tag="pss")
                        for j in range(jp, jp + hp):
                            nc.tensor.matmul(
                                ps[:, j - jp, :nq],
                                _mm(k[32 * j:32 * j + D, kc * KCH:(kc + 1) * KCH]),
                                _mm(q[32 * j:32 * j + D, :]),
                                start=True, stop=True,
                                tile_position=(32 * j, 0),
                            )
                        p = sbpp.tile([128, hp, 512], F32, tag="sbp")
                        nc.scalar.activation(
                            p[:, :, :nq], ps[:, :, :nq],
                            mybir.ActivationFunctionType.Exp,
                            bias=0.0, scale=SCALE,
                        )
                        for j in range(jp, jp + hp):
                            h = HPG * g + j
                            # col-tiled accumulation chains touch disjoint
                            # partition ranges (32j..32j+20) of one bank; the
                            # sim's zero-region check is bank-granular, so
                            # bypass it.
                            nc.tensor.matmul(
                                po[32 * j:32 * j + 32, :],
                                _mm(sbV[kc][:, 21 * h:21 * h + 32]),
                                _mm(p[:, j - jp, :nq]),
                                start=(kc == 0), stop=(kc == nkc - 1),
                                tile_position=(0, 32 * j),
                                skip_group_check=True,
                            )

                o = sbop.tile([128, nq], F32, tag="sbo")
                nc.vector.tensor_copy(o[:], po[:])
                for qc in range(nqc):
                    pt = pst.tile([128, 128], F32, tag="pst")
                    nc.tensor.transpose(pt[:], o[:, qc * QCH:(qc + 1) * QCH], ident[:])
                    ptb = pt.rearrange("p (j c) -> p j c", j=HPG)  # [128,4,32]
                    r = sbrp.tile([128, HPG], F32, tag="sbr")
                    nc.vector.reciprocal(r[:], ptb[:, :, D])
                    nc.vector.tensor_mul(
                        asms[qc][:, g * 80:(g + 1) * 80]
                            .rearrange("p (j d) -> p j d", j=HPG),
                        ptb[:, :, 0:D],
                        r.unsqueeze(2).broadcast_to([128, HPG, D]),
                    )

            for qc in range(nqc):
                nc.sync.dma_start(
                    dr[f"o{s}"][qc * QCH:(qc + 1) * QCH, :], asms[qc][:]
                )


def _build_nc(slots):
    nc = bacc.Bacc(
        "TRN2",
        target_bir_lowering=False,
        debug=False,
        enable_asserts=False,
        num_devices=N_CORES,
    )
    dr = {}
    for s, (nq, nkc, _grp) in enumerate(slots):
        nkv = nkc * KCH
        dr[f"qt{s}"] = nc.dram_tensor(f"qt{s}", [D + 1, nq], F32, kind="ExternalInput").ap()
        dr[f"kt{s}"] = nc.dram_tensor(f"kt{s}", [D + 1, nkv], F32, kind="ExternalInput").ap()
        dr[f"vt{s}"] = nc.dram_tensor(f"vt{s}", [D + 1, nkv], F32, kind="ExternalInput").ap()
        dr[f"o{s}"] = nc.dram_tensor(f"o{s}", [nq, OUT_DIM], F32, kind="ExternalOutput").ap()
    dr["wq"] = nc.dram_tensor("wq", [D + 1, NG * 128], F32, kind="ExternalInput").ap()
    dr["wk"] = nc.dram_tensor("wk", [D + 1, NG * 128], F32, kind="ExternalInput").ap()
    dr["wv"] = nc.dram_tensor("wv", [D + 1, VW], F32, kind="ExternalInput").ap()
    dr["ident"] = nc.dram_tensor("ident", [128, 128], F32, kind="ExternalInput").ap()

    with tile.TileContext(nc) as tc:
        _emit(tc, nc, dr, slots)
    nc.compile()
    return nc


# ----------------------------------------------------------------- driver

def kernel(**inputs):
    global LAST_RESULT
    Q_seq = np.ascontiguousarray(np.asarray(inputs["Q_seq"], dtype=np.float32))
    K_seq = np.ascontiguousarray(np.asarray(inputs["K_seq"], dtype=np.float32))
    V_seq = np.ascontiguousarray(np.asarray(inputs["V_seq"], dtype=np.float32))
    Q_len = np.asarray(inputs["Q_len"]).reshape(-1).astype(np.int64)
    V_len = np.asarray(inputs["V_len"]).reshape(-1).astype(np.int64)
    WQ_w = np.asarray(inputs["WQ_w"], dtype=np.float32)
    WQ_b = np.asarray(inputs["WQ_b"], dtype=np.float32)
    WK_w = np.asarray(inputs["WK_w"], dtype=np.float32)
    WK_b = np.asarray(inputs["WK_b"], dtype=np.float32)
    WV_w = np.asarray(inputs["WV_w"], dtype=np.float32)
    WV_b = np.asarray(inputs["WV_b"], dtype=np.float32)

    slots = _plan(Q_len, V_len)
    nc = _build_nc(slots)

    wq = _pack_qk_weights(WQ_w, WQ_b)
    wk = _pack_qk_weights(WK_w, WK_b)
    wv = _pack_v_weights(WV_w, WV_b)
    ident = np.eye(128, dtype=np.float32)

    in_maps = []
    for c in range(N_CORES):
        m = {"wq": wq, "wk": wk, "wv": wv, "ident": ident}
        for s, (nq, nkc, grp) in enumerate(slots):
            b = grp[c]
            nkv = nkc * KCH
            m[f"qt{s}"] = _prep_qt(Q_seq[b], nq)
            m[f"kt{s}"] = _prep_kvt(K_seq[b], V_len[b], nkv)
            m[f"vt{s}"] = _prep_kvt(V_seq[b], V_len[b], nkv)
        in_maps.append(m)

    res = run_bass_kernel_spmd(
        nc, in_maps, core_ids=list(range(N_CORES)), trace=TRACE
    )
    LAST_RESULT = res

    out = np.zeros((B, LQ, OUT_DIM), np.float32)
    for c in range(N_CORES):
        for s, (_nq, _nkc, grp) in enumerate(slots):
            b = grp[c]
            ql = int(Q_len[b])
            if ql > 0:
                out[b, :ql] = res.results[c][f"o{s}"][:ql]
    return out


# revision 15
# speedup vs baseline: 1.4766x; 1.4766x over previous
"""Masked multi-head attention (B=32, Lq=Lk=512, H=20, D=20) on 8 TRN2 NeuronCores.

Strategy:
  - Data-parallel over batch: 32 batches -> 8 cores x 4 "slots" (SPMD: one NEFF).
  - Host bakes per-slot static shapes (nq = padded Q_len, nkc = kv chunks from
    V_len) and bin-packs batches into slot groups to minimize padded work.
  - Host pre-transposes sequences to [21, L] (20 features + ones row).  The
    ones row realizes: exact linear bias, zeroing of masked kv positions
    (mask folded into V/K inputs), and a free softmax-denominator column in
    the projected V tile.
  - Device per (slot, head-group of 4 heads at 32-partition offsets):
      proj Q/K/V (PE, contraction 21)
      S^T = K_h @ Q_h^T   row-tiled 4 heads concurrently  (PSUM)
      P^T = exp(S^T / sqrt(D))  one ACTIVATE per pack     (SBUF)
      O^T(+sums row) = [V_h|mask]^T @ P^T  col-tiled, accumulated over kv chunks
      PE transpose -> [q, .] layout; DVE reciprocal + broadcast multiply
      assemble [128, 400] and DMA to DRAM.
  - Host scatters per-slot outputs into the final [32, 512, 400] (rows beyond
    Q_len stay zero, which implements the multiplicative q mask exactly).
"""

import math
import random

import numpy as np

import concourse.bacc as bacc
import concourse.bass as bass
import concourse.tile as tile
from concourse import mybir
from concourse.bass_utils import run_bass_kernel_spmd

B, LQ, LK = 32, 512, 512
H, D = 20, 20
OUT_DIM = H * D  # 400
N_CORES = 8
N_SLOTS = B // N_CORES  # 4
QCH = 128
KCH = 128
NG = 5  # head groups
HPG = 4  # heads per group (at partition offsets 0/32/64/96)
VW = H * 21 + 12  # 432 (even, for fp32r): per-head 20 dims + 1 ones col,
                  # padded so a 32-wide lhsT slice exists for every head
SCALE = 1.0 / math.sqrt(D)

F32 = mybir.dt.float32

# Perf knobs
USE_F32R = True  # bitcast matmul operands to float32r (fast fp32 path)
TRACE = False  # set True to capture NTFF profile (slower)
LAST_RESULT = None  # BassKernelResults of the last run (for test harness)


# ----------------------------------------------------------------- planning

def _plan(q_len, v_len):
    """Group 32 batches into N_SLOTS groups of N_CORES, minimizing baked cost.

    Returns list of (nq, nkc, batches[8]) sorted big->small."""
    nqc = [max(1, math.ceil(min(int(q), LQ) / QCH)) for q in q_len]
    kv_eff = [LK if int(v) <= 0 else min(int(v), LK) for v in v_len]
    nkc = [math.ceil(k / KCH) for k in kv_eff]
    cost = [a * b for a, b in zip(nqc, nkc)]
    order = sorted(range(B), key=lambda b: -cost[b])

    def baked(gs):
        t = 0
        for g in gs:
            if g:
                t += max(nqc[b] for b in g) * max(nkc[b] for b in g)
        return t

    groups = [[] for _ in range(N_SLOTS)]
    for b in order:
        best, bestc = None, None
        for gi in range(N_SLOTS):
            if len(groups[gi]) >= N_CORES:
                continue
            groups[gi].append(b)
            c = baked(groups)
            groups[gi].pop()
            if bestc is None or c < bestc:
                best, bestc = gi, c
        groups[best].append(b)
    rng = random.Random(0)
    cur = baked(groups)
    for _ in range(6000):
        g1, g2 = rng.randrange(N_SLOTS), rng.randrange(N_SLOTS)
        if g1 == g2:
            continue
        i1, i2 = rng.randrange(N_CORES), rng.randrange(N_CORES)
        groups[g1][i1], groups[g2][i2] = groups[g2][i2], groups[g1][i1]
        c = baked(groups)
        if c <= cur:
            cur = c
        else:
            groups[g1][i1], groups[g2][i2] = groups[g2][i2], groups[g1][i1]
    slots = []
    for g in groups:
        snq = max(nqc[b] for b in g) * QCH
        snkc = max(nkc[b] for b in g)
        slots.append((snq, snkc, list(g)))
    slots.sort(key=lambda s: -(s[0] * s[1]))
    return slots


# ------------------------------------------------------------ host packing

def _pack_qk_weights(W, bias):
    """[400, 20] linear weight -> [21, NG*128] lhsT layout (head 4g+j at
    columns 128g+32j .. +20; row 20 = bias)."""
    t = np.zeros((D + 1, NG * 128), np.float32)
    for h in range(H):
        g, j = divmod(h, HPG)
        c = g * 128 + 32 * j
        t[:D, c:c + D] = W[h * D:(h + 1) * D, :].T
        t[D, c:c + D] = bias[h * D:(h + 1) * D]
    return t


def _pack_v_weights(W, bias):
    """[400, 20] -> [21, 420] rhs layout: head h at cols 21h..21h+19,
    ones-generator col at 21h+20."""
    t = np.zeros((D + 1, VW), np.float32)
    for h in range(H):
        c = 21 * h
        t[:D, c:c + D] = W[h * D:(h + 1) * D, :].T
        t[D, c:c + D] = bias[h * D:(h + 1) * D]
        t[D, c + D] = 1.0
    return t


def _prep_qt(qs, nq):
    t = np.zeros((D + 1, nq), np.float32)
    n = min(nq, LQ)
    t[:D, :n] = qs[:n].T
    t[D, :n] = 1.0
    return t


def _prep_kvt(ks, vlen, nkv):
    """K/V sequence transposed with ones row; columns >= V_len zeroed
    (vlen==0 means "uniform -1e12 shift" in the reference == full attention)."""
    t = np.zeros((D + 1, nkv), np.float32)
    n = min(nkv, LK) if int(vlen) <= 0 else min(nkv, int(vlen))
    t[:D, :n] = ks[:n].T
    t[D, :n] = 1.0
    return t


# ------------------------------------------------------------ device build

def _emit(tc, nc, dr, slots):
    # fp32r matmul operands must come from instructions that round to fp32r;
    # DMA can't, so DMA'd tensors get one DVE rounding copy each.
    DT = mybir.dt.float32r if USE_F32R else F32
    with (
        tc.tile_pool(name="wpool", bufs=1) as wpool,
        tc.tile_pool(name="seqin", bufs=2) as seqp,
        tc.tile_pool(name="sbq", bufs=3) as sbqp,
        tc.tile_pool(name="sbk", bufs=3) as sbkp,
        tc.tile_pool(name="sbv", bufs=6) as sbvp,
        tc.tile_pool(name="sbp", bufs=4) as sbpp,
        tc.tile_pool(name="sbo", bufs=2) as sbop,
        tc.tile_pool(name="sbr", bufs=4) as sbrp,
        tc.tile_pool(name="asm", bufs=6) as asmp,
        tc.tile_pool(name="ppj", bufs=2, space="PSUM") as ppj,
        tc.tile_pool(name="pss", bufs=2, space="PSUM") as pss,
        tc.tile_pool(name="pso", bufs=1, space="PSUM") as pso,
        tc.tile_pool(name="pst", bufs=1, space="PSUM") as pst,
    ):
        def load_rounded(name, shape, pool, tag):
            raw = pool.tile(shape, F32, tag=tag + "_raw", name=name + "_raw")
            nc.sync.dma_start(raw[:], dr[name])
            if not USE_F32R:
                return raw
            t = pool.tile(shape, DT, tag=tag, name=name + "_r")
            nc.vector.tensor_copy(t[:], raw[:])
            return t

        wq = load_rounded("wq", [D + 1, NG * 128], wpool, "wq")
        wk = load_rounded("wk", [D + 1, NG * 128], wpool, "wk")
        wv = load_rounded("wv", [D + 1, VW], wpool, "wv")
        ident = wpool.tile([128, 128], F32, tag="ident")
        nc.sync.dma_start(ident[:], dr["ident"])

        for s, (nq, nkc, _g) in enumerate(slots):
            nkv = nkc * KCH
            nqc = nq // QCH
            # 2 heads per S^T psum tile; each head's [128, nq] slice padded to a
            # full 2KB bank so no two matmul outputs share a PSUM zero region.
            hp = 2

            qt = load_rounded(f"qt{s}", [D + 1, nq], seqp, "qt")
            kt = load_rounded(f"kt{s}", [D + 1, nkv], seqp, "kt")
            vt = load_rounded(f"vt{s}", [D + 1, nkv], seqp, "vt")

            # V projection: per kv chunk -> [128, 420] (incl. masked ones cols)
            sbV = []
            for kc in range(nkc):
                pv = ppj.tile([128, 512], F32, tag="ppj")
                nc.tensor.matmul(
                    pv[:, :VW], vt[:, kc * KCH:(kc + 1) * KCH], wv[:],
                    start=True, stop=True,
                )
                v = sbvp.tile([128, VW], F32, tag="sbv")
                nc.vector.tensor_copy(v[:], pv[:, :VW])
                sbV.append(v)

            asms = [
                asmp.tile([128, OUT_DIM], F32, tag="asm", name=f"asm{s}_{qc}")
                for qc in range(nqc)
            ]

            for g in range(NG):
                pq = ppj.tile([128, 512], F32, tag="ppj")
                nc.tensor.matmul(
                    pq[:, :nq], wq[:, g * 128:(g + 1) * 128], qt[:],
                    start=True, stop=True,
                )
                q = sbqp.tile([128, nq], DT, tag="sbq")
                nc.vector.tensor_copy(q[:], pq[:, :nq])

                pk = ppj.tile([128, 512], F32, tag="ppj")
                nc.tensor.matmul(
                    pk[:, :nkv], wk[:, g * 128:(g + 1) * 128], kt[:],
                    start=True, stop=True,
                )
                k = sbkp.tile([128, nkv], DT, tag="sbk")
                nc.vector.tensor_copy(k[:], pk[:, :nkv])

                po = pso.tile([128, nq], F32, tag="pso")

                for kc in range(nkc):
                    for jp in range(0, HPG, hp):
                        ps = pss.tile([128, hp, 512], F32, tag="pss")
                        for j in range(jp, jp + hp):
                            nc.tensor.matmul(
                                ps[:, j - jp, :nq],
                                k[32 * j:32 * j + D, kc * KCH:(kc + 1) * KCH],
                                q[32 * j:32 * j + D, :],
                                start=True, stop=True,
                                tile_position=(32 * j, 0),
                            )
                        p = sbpp.tile([128, hp, 512], F32, tag="sbp")
                        nc.scalar.activation(
                            p[:, :, :nq], ps[:, :, :nq],
                            mybir.ActivationFunctionType.Exp,
                            bias=0.0, scale=SCALE,
                        )
                        for j in range(jp, jp + hp):
                            h = HPG * g + j
                            # col-tiled accumulation chains touch disjoint
                            # partition ranges (32j..32j+20) of one bank; the
                            # sim's zero-region check is bank-granular, so
                            # bypass it.
                            nc.tensor.matmul(
                                po[32 * j:32 * j + 32, :],
                                sbV[kc][:, 21 * h:21 * h + 32],
                                p[:, j - jp, :nq],
                                start=(kc == 0), stop=(kc == nkc - 1),
                                tile_position=(0, 32 * j),
                                skip_group_check=True,
                            )

                o = sbop.tile([128, nq], F32, tag="sbo")
                nc.vector.tensor_copy(o[:], po[:])
                for qc in range(nqc):
                    pt = pst.tile([128, 128], F32, tag="pst")
                    nc.tensor.transpose(pt[:], o[:, qc * QCH:(qc + 1) * QCH], ident[:])
                    ptb = pt.rearrange("p (j c) -> p j c", j=HPG)  # [128,4,32]
                    r = sbrp.tile([128, HPG], F32, tag="sbr")
                    nc.vector.reciprocal(r[:], ptb[:, :, D])
                    nc.vector.tensor_mul(
                        asms[qc][:, g * 80:(g + 1) * 80]
                            .rearrange("p (j d) -> p j d", j=HPG),
                        ptb[:, :, 0:D],
                        r.unsqueeze(2).broadcast_to([128, HPG, D]),
                    )

            for qc in range(nqc):
                nc.sync.dma_start(
                    dr[f"o{s}"][qc * QCH:(qc + 1) * QCH, :], asms[qc][:]
                )


def _build_nc(slots):
    nc = bacc.Bacc(
        "TRN2",
        target_bir_lowering=False,
        debug=False,
        enable_asserts=False,
        num_devices=N_CORES,
    )
    dr = {}
    for s, (nq, nkc, _grp) in enumerate(slots):
        nkv = nkc * KCH
        dr[f"qt{s}"] = nc.dram_tensor(f"qt{s}", [D + 1, nq], F32, kind="ExternalInput").ap()
        dr[f"kt{s}"] = nc.dram_tensor(f"kt{s}", [D + 1, nkv], F32, kind="ExternalInput").ap()
        dr[f"vt{s}"] = nc.dram_tensor(f"vt{s}", [D + 1, nkv], F32, kind="ExternalInput").ap()
        dr[f"o{s}"] = nc.dram_tensor(f"o{s}", [nq, OUT_DIM], F32, kind="ExternalOutput").ap()
    dr["wq"] = nc.dram_tensor("wq", [D + 1, NG * 128], F32, kind="ExternalInput").ap()
    dr["wk"] = nc.dram_tensor("wk", [D + 1, NG * 128], F32, kind="ExternalInput").ap()
    dr["wv"] = nc.dram_tensor("wv", [D + 1, VW], F32, kind="ExternalInput").ap()
    dr["ident"] = nc.dram_tensor("ident", [128, 128], F32, kind="ExternalInput").ap()

    with tile.TileContext(nc) as tc:
        _emit(tc, nc, dr, slots)
    nc.compile()
    return nc


# ----------------------------------------------------------------- driver

def kernel(**inputs):
    global LAST_RESULT
    Q_seq = np.ascontiguousarray(np.asarray(inputs["Q_seq"], dtype=np.float32))
    K_seq = np.ascontiguousarray(np.asarray(inputs["K_seq"], dtype=np.float32))
    V_seq = np.ascontiguousarray(np.asarray(inputs["V_seq"], dtype=np.float32))
    Q_len = np.asarray(inputs["Q_len"]).reshape(-1).astype(np.int64)
    V_len = np.asarray(inputs["V_len"]).reshape(-1).astype(np.int64)
    WQ_w = np.asarray(inputs["WQ_w"], dtype=np.float32)
    WQ_b = np.asarray(inputs["WQ_b"], dtype=np.float32)
    WK_w = np.asarray(inputs["WK_w"], dtype=np.float32)
    WK_b = np.asarray(inputs["WK_b"], dtype=np.float32)
    WV_w = np.asarray(inputs["WV_w"], dtype=np.float32)
    WV_b = np.asarray(inputs["WV_b"], dtype=np.float32)

    slots = _plan(Q_len, V_len)
    nc = _build_nc(slots)

    wq = _pack_qk_weights(WQ_w, WQ_b)
    wk = _pack_qk_weights(WK_w, WK_b)
    wv = _pack_v_weights(WV_w, WV_b)
    ident = np.eye(128, dtype=np.float32)

    in_maps = []
    for c in range(N_CORES):
        m = {"wq": wq, "wk": wk, "wv": wv, "ident": ident}
        for s, (nq, nkc, grp) in enumerate(slots):
            b = grp[c]
            nkv = nkc * KCH
            m[f"qt{s}"] = _prep_qt(Q_seq[b], nq)
            m[f"kt{s}"] = _prep_kvt(K_seq[b], V_len[b], nkv)
            m[f"vt{s}"] = _prep_kvt(V_seq[b], V_len[b], nkv)
        in_maps.append(m)

    res = run_bass_kernel_spmd(
        nc, in_maps, core_ids=list(range(N_CORES)), trace=TRACE
    )
    LAST_RESULT = res

    out = np.zeros((B, LQ, OUT_DIM), np.float32)
    for c in range(N_CORES):
        for s, (_nq, _nkc, grp) in enumerate(slots):
            b = grp[c]
            ql = int(Q_len[b])
            if ql > 0:
                out[b, :ql] = res.results[c][f"o{s}"][:ql]
    return out


# revision 16
# speedup vs baseline: 1.9087x; 1.2926x over previous
"""Masked multi-head attention (B=32, Lq=Lk=512, H=20, D=20) on 8 TRN2 NeuronCores.

Strategy:
  - Data-parallel over batch: 32 batches -> 8 cores x 4 "slots" (SPMD: one NEFF).
  - Host bakes per-slot static shapes (nq = padded Q_len, nkc = kv chunks from
    V_len) and bin-packs batches into slot groups to minimize padded work.
  - Host pre-transposes sequences to [21, L] (20 features + ones row).  The
    ones row realizes: exact linear bias, zeroing of masked kv positions
    (mask folded into V/K inputs), and a free softmax-denominator column in
    the projected V tile.
  - Device per (slot, head-group of 4 heads at 32-partition offsets):
      proj Q/K/V (PE, contraction 21)
      S^T = K_h @ Q_h^T   row-tiled 4 heads concurrently  (PSUM)
      P^T = exp(S^T / sqrt(D))  one ACTIVATE per pack     (SBUF)
      O^T(+sums row) = [V_h|mask]^T @ P^T  col-tiled, accumulated over kv chunks
      PE transpose -> [q, .] layout; DVE reciprocal + broadcast multiply
      assemble [128, 400] and DMA to DRAM.
  - Host scatters per-slot outputs into the final [32, 512, 400] (rows beyond
    Q_len stay zero, which implements the multiplicative q mask exactly).
"""

import math
import random

import numpy as np

import concourse.bacc as bacc
import concourse.bass as bass
import concourse.tile as tile
from concourse import mybir
from concourse.bass_utils import run_bass_kernel_spmd

B, LQ, LK = 32, 512, 512
H, D = 20, 20
OUT_DIM = H * D  # 400
N_CORES = 8
N_SLOTS = B // N_CORES  # 4
QCH = 128
KCH = 128
NG = 5  # head groups
HPG = 4  # heads per group (at partition offsets 0/32/64/96)
VW = H * 21 + 12  # 432 (even, for fp32r): per-head 20 dims + 1 ones col,
                  # padded so a 32-wide lhsT slice exists for every head
SCALE = 1.0 / math.sqrt(D)

F32 = mybir.dt.float32

# Perf knobs
USE_F32R = True  # bitcast matmul operands to float32r (fast fp32 path)
TRACE = False  # set True to capture NTFF profile (slower)
LAST_RESULT = None  # BassKernelResults of the last run (for test harness)


# ----------------------------------------------------------------- planning

def _plan(q_len, v_len):
    """Group 32 batches into N_SLOTS groups of N_CORES, minimizing baked cost.

    Returns list of (nq, nkc, batches[8]) sorted big->small."""
    nqc = [max(1, math.ceil(min(int(q), LQ) / QCH)) for q in q_len]
    kv_eff = [LK if int(v) <= 0 else min(int(v), LK) for v in v_len]
    nkc = [math.ceil(k / KCH) for k in kv_eff]
    cost = [a * b for a, b in zip(nqc, nkc)]
    order = sorted(range(B), key=lambda b: -cost[b])

    def baked(gs):
        t = 0
        for g in gs:
            if g:
                t += max(nqc[b] for b in g) * max(nkc[b] for b in g)
        return t

    groups = [[] for _ in range(N_SLOTS)]
    for b in order:
        best, bestc = None, None
        for gi in range(N_SLOTS):
            if len(groups[gi]) >= N_CORES:
                continue
            groups[gi].append(b)
            c = baked(groups)
            groups[gi].pop()
            if bestc is None or c < bestc:
                best, bestc = gi, c
        groups[best].append(b)
    rng = random.Random(0)
    cur = baked(groups)
    for _ in range(6000):
        g1, g2 = rng.randrange(N_SLOTS), rng.randrange(N_SLOTS)
        if g1 == g2:
            continue
        i1, i2 = rng.randrange(N_CORES), rng.randrange(N_CORES)
        groups[g1][i1], groups[g2][i2] = groups[g2][i2], groups[g1][i1]
        c = baked(groups)
        if c <= cur:
            cur = c
        else:
            groups[g1][i1], groups[g2][i2] = groups[g2][i2], groups[g1][i1]
    slots = []
    for g in groups:
        snq = max(nqc[b] for b in g) * QCH
        snkc = max(nkc[b] for b in g)
        slots.append((snq, snkc, list(g)))
    slots.sort(key=lambda s: -(s[0] * s[1]))
    return slots


# ------------------------------------------------------------ host packing

def _pack_qk_weights(W, bias):
    """[400, 20] linear weight -> [21, NG*128] lhsT layout (head 4g+j at
    columns 128g+32j .. +20; row 20 = bias)."""
    t = np.zeros((D + 1, NG * 128), np.float32)
    for h in range(H):
        g, j = divmod(h, HPG)
        c = g * 128 + 32 * j
        t[:D, c:c + D] = W[h * D:(h + 1) * D, :].T
        t[D, c:c + D] = bias[h * D:(h + 1) * D]
    return t


def _pack_v_weights(W, bias):
    """[400, 20] -> [21, 420] rhs layout: head h at cols 21h..21h+19,
    ones-generator col at 21h+20."""
    t = np.zeros((D + 1, VW), np.float32)
    for h in range(H):
        c = 21 * h
        t[:D, c:c + D] = W[h * D:(h + 1) * D, :].T
        t[D, c:c + D] = bias[h * D:(h + 1) * D]
        t[D, c + D] = 1.0
    return t


def _prep_qt(qs, nq):
    t = np.zeros((D + 1, nq), np.float32)
    n = min(nq, LQ)
    t[:D, :n] = qs[:n].T
    t[D, :n] = 1.0
    return t


def _prep_kvt(ks, vlen, nkv):
    """K/V sequence transposed with ones row; columns >= V_len zeroed
    (vlen==0 means "uniform -1e12 shift" in the reference == full attention)."""
    t = np.zeros((D + 1, nkv), np.float32)
    n = min(nkv, LK) if int(vlen) <= 0 else min(nkv, int(vlen))
    t[:D, :n] = ks[:n].T
    t[D, :n] = 1.0
    return t


# ------------------------------------------------------------ device build

def _emit(tc, nc, dr, slots):
    # fp32r matmul operands must come from instructions that round to fp32r;
    # DMA can't, so DMA'd tensors get one DVE rounding copy each.
    DT = mybir.dt.float32r if USE_F32R else F32
    with (
        tc.tile_pool(name="wpool", bufs=1) as wpool,
        tc.tile_pool(name="seqin", bufs=2) as seqp,
        tc.tile_pool(name="sbq", bufs=3) as sbqp,
        tc.tile_pool(name="sbk", bufs=3) as sbkp,
        tc.tile_pool(name="sbv", bufs=6) as sbvp,
        tc.tile_pool(name="sbp", bufs=4) as sbpp,
        tc.tile_pool(name="sbo", bufs=2) as sbop,
        tc.tile_pool(name="sbr", bufs=4) as sbrp,
        tc.tile_pool(name="asm", bufs=6) as asmp,
        tc.tile_pool(name="ppj", bufs=2, space="PSUM") as ppj,
        tc.tile_pool(name="pss", bufs=2, space="PSUM") as pss,
        tc.tile_pool(name="pso", bufs=1, space="PSUM") as pso,
        tc.tile_pool(name="pst", bufs=1, space="PSUM") as pst,
    ):
        def load_rounded(name, shape, pool, tag):
            raw = pool.tile(shape, F32, tag=tag + "_raw", name=name + "_raw")
            nc.sync.dma_start(raw[:], dr[name])
            if not USE_F32R:
                return raw
            t = pool.tile(shape, DT, tag=tag, name=name + "_r")
            nc.vector.tensor_copy(t[:], raw[:])
            return t

        wq = load_rounded("wq", [D + 1, NG * 128], wpool, "wq")
        wk = load_rounded("wk", [D + 1, NG * 128], wpool, "wk")
        wv = load_rounded("wv", [D + 1, VW], wpool, "wv")
        ident = wpool.tile([128, 128], F32, tag="ident")
        nc.sync.dma_start(ident[:], dr["ident"])

        for s, (nq, nkc, _g) in enumerate(slots):
            nkv = nkc * KCH
            nqc = nq // QCH
            # 2 heads per S^T psum tile; each head's [128, nq] slice padded to a
            # full 2KB bank so no two matmul outputs share a PSUM zero region.
            hp = 2

            qt = load_rounded(f"qt{s}", [D + 1, nq], seqp, "qt")
            kt = load_rounded(f"kt{s}", [D + 1, nkv], seqp, "kt")
            vt = load_rounded(f"vt{s}", [D + 1, nkv], seqp, "vt")

            # V projection: per kv chunk -> [128, 420] (incl. masked ones cols)
            sbV = []
            for kc in range(nkc):
                pv = ppj.tile([128, 512], F32, tag="ppj")
                nc.tensor.matmul(
                    pv[:, :VW], vt[:, kc * KCH:(kc + 1) * KCH], wv[:],
                    start=True, stop=True,
                )
                v = sbvp.tile([128, VW], mybir.dt.bfloat16, tag="sbv")
                nc.vector.tensor_copy(v[:], pv[:, :VW])
                sbV.append(v)

            asms = [
                asmp.tile([128, OUT_DIM], F32, tag="asm", name=f"asm{s}_{qc}")
                for qc in range(nqc)
            ]

            for g in range(NG):
                pq = ppj.tile([128, 512], F32, tag="ppj")
                nc.tensor.matmul(
                    pq[:, :nq], wq[:, g * 128:(g + 1) * 128], qt[:],
                    start=True, stop=True,
                )
                q = sbqp.tile([128, nq], DT, tag="sbq")
                nc.vector.tensor_copy(q[:], pq[:, :nq])

                pk = ppj.tile([128, 512], F32, tag="ppj")
                nc.tensor.matmul(
                    pk[:, :nkv], wk[:, g * 128:(g + 1) * 128], kt[:],
                    start=True, stop=True,
                )
                k = sbkp.tile([128, nkv], DT, tag="sbk")
                nc.vector.tensor_copy(k[:], pk[:, :nkv])

                po = pso.tile([128, nq], F32, tag="pso")

                for kc in range(nkc):
                    for jp in range(0, HPG, hp):
                        ps = pss.tile([128, hp, 512], F32, tag="pss")
                        for j in range(jp, jp + hp):
                            nc.tensor.matmul(
                                ps[:, j - jp, :nq],
                                k[32 * j:32 * j + D, kc * KCH:(kc + 1) * KCH],
                                q[32 * j:32 * j + D, :],
                                start=True, stop=True,
                                tile_position=(32 * j, 0),
                            )
                        p = sbpp.tile([128, hp, 512], mybir.dt.bfloat16, tag="sbp")
                        nc.scalar.activation(
                            p[:, :, :nq], ps[:, :, :nq],
                            mybir.ActivationFunctionType.Exp,
                            bias=0.0, scale=SCALE,
                        )
                        for j in range(jp, jp + hp):
                            h = HPG * g + j
                            # col-tiled accumulation chains touch disjoint
                            # partition ranges (32j..32j+20) of one bank; the
                            # sim's zero-region check is bank-granular, so
                            # bypass it.
                            nc.tensor.matmul(
                                po[32 * j:32 * j + 32, :],
                                sbV[kc][:, 21 * h:21 * h + 32],
                                p[:, j - jp, :nq],
                                start=(kc == 0), stop=(kc == nkc - 1),
                                tile_position=(0, 32 * j),
                                skip_group_check=True,
                            )

                o = sbop.tile([128, nq], F32, tag="sbo")
                nc.vector.tensor_copy(o[:], po[:])
                for qc in range(nqc):
                    pt = pst.tile([128, 128], F32, tag="pst")
                    nc.tensor.transpose(pt[:], o[:, qc * QCH:(qc + 1) * QCH], ident[:])
                    ptb = pt.rearrange("p (j c) -> p j c", j=HPG)  # [128,4,32]
                    r = sbrp.tile([128, HPG], F32, tag="sbr")
                    nc.vector.reciprocal(r[:], ptb[:, :, D])
                    nc.vector.tensor_mul(
                        asms[qc][:, g * 80:(g + 1) * 80]
                            .rearrange("p (j d) -> p j d", j=HPG),
                        ptb[:, :, 0:D],
                        r.unsqueeze(2).broadcast_to([128, HPG, D]),
                    )

            for qc in range(nqc):
                nc.sync.dma_start(
                    dr[f"o{s}"][qc * QCH:(qc + 1) * QCH, :], asms[qc][:]
                )


def _build_nc(slots):
    nc = bacc.Bacc(
        "TRN2",
        target_bir_lowering=False,
        debug=False,
        enable_asserts=False,
        num_devices=N_CORES,
    )
    dr = {}
    for s, (nq, nkc, _grp) in enumerate(slots):
        nkv = nkc * KCH
        dr[f"qt{s}"] = nc.dram_tensor(f"qt{s}", [D + 1, nq], F32, kind="ExternalInput").ap()
        dr[f"kt{s}"] = nc.dram_tensor(f"kt{s}", [D + 1, nkv], F32, kind="ExternalInput").ap()
        dr[f"vt{s}"] = nc.dram_tensor(f"vt{s}", [D + 1, nkv], F32, kind="ExternalInput").ap()
        dr[f"o{s}"] = nc.dram_tensor(f"o{s}", [nq, OUT_DIM], F32, kind="ExternalOutput").ap()
    dr["wq"] = nc.dram_tensor("wq", [D + 1, NG * 128], F32, kind="ExternalInput").ap()
    dr["wk"] = nc.dram_tensor("wk", [D + 1, NG * 128], F32, kind="ExternalInput").ap()
    dr["wv"] = nc.dram_tensor("wv", [D + 1, VW], F32, kind="ExternalInput").ap()
    dr["ident"] = nc.dram_tensor("ident", [128, 128], F32, kind="ExternalInput").ap()

    with tile.TileContext(nc) as tc:
        _emit(tc, nc, dr, slots)
    nc.compile()
    return nc


# ----------------------------------------------------------------- driver

def kernel(**inputs):
    global LAST_RESULT
    Q_seq = np.ascontiguousarray(np.asarray(inputs["Q_seq"], dtype=np.float32))
    K_seq = np.ascontiguousarray(np.asarray(inputs["K_seq"], dtype=np.float32))
    V_seq = np.ascontiguousarray(np.asarray(inputs["V_seq"], dtype=np.float32))
    Q_len = np.asarray(inputs["Q_len"]).reshape(-1).astype(np.int64)
    V_len = np.asarray(inputs["V_len"]).reshape(-1).astype(np.int64)
    WQ_w = np.asarray(inputs["WQ_w"], dtype=np.float32)
    WQ_b = np.asarray(inputs["WQ_b"], dtype=np.float32)
    WK_w = np.asarray(inputs["WK_w"], dtype=np.float32)
    WK_b = np.asarray(inputs["WK_b"], dtype=np.float32)
    WV_w = np.asarray(inputs["WV_w"], dtype=np.float32)
    WV_b = np.asarray(inputs["WV_b"], dtype=np.float32)

    slots = _plan(Q_len, V_len)
    nc = _build_nc(slots)

    wq = _pack_qk_weights(WQ_w, WQ_b)
    wk = _pack_qk_weights(WK_w, WK_b)
    wv = _pack_v_weights(WV_w, WV_b)
    ident = np.eye(128, dtype=np.float32)

    in_maps = []
    for c in range(N_CORES):
        m = {"wq": wq, "wk": wk, "wv": wv, "ident": ident}
        for s, (nq, nkc, grp) in enumerate(slots):
            b = grp[c]
            nkv = nkc * KCH
            m[f"qt{s}"] = _prep_qt(Q_seq[b], nq)
            m[f"kt{s}"] = _prep_kvt(K_seq[b], V_len[b], nkv)
            m[f"vt{s}"] = _prep_kvt(V_seq[b], V_len[b], nkv)
        in_maps.append(m)

    res = run_bass_kernel_spmd(
        nc, in_maps, core_ids=list(range(N_CORES)), trace=TRACE
    )
    LAST_RESULT = res

    out = np.zeros((B, LQ, OUT_DIM), np.float32)
    for c in range(N_CORES):
        for s, (_nq, _nkc, grp) in enumerate(slots):
            b = grp[c]
            ql = int(Q_len[b])
            if ql > 0:
                out[b, :ql] = res.results[c][f"o{s}"][:ql]
    return out


# revision 19
# speedup vs baseline: 1.9623x; 1.0281x over previous
"""Masked multi-head attention (B=32, Lq=Lk=512, H=20, D=20) on 8 TRN2 NeuronCores.

Strategy:
  - Data-parallel over batch: 32 batches -> 8 cores x 4 "slots" (SPMD: one NEFF).
  - Host bakes per-slot static shapes (nq = padded Q_len, nkc = kv chunks from
    V_len) and bin-packs batches into slot groups to minimize padded work.
  - Host pre-transposes sequences to [21, L] (20 features + ones row).  The
    ones row realizes: exact linear bias, zeroing of masked kv positions
    (mask folded into V/K inputs), and a free softmax-denominator column in
    the projected V tile.
  - Device per (slot, head-group of 4 heads at 32-partition offsets):
      proj Q/K/V (PE, contraction 21)
      S^T = K_h @ Q_h^T   row-tiled 4 heads concurrently  (PSUM)
      P^T = exp(S^T / sqrt(D))  one ACTIVATE per pack     (SBUF)
      O^T(+sums row) = [V_h|mask]^T @ P^T  col-tiled, accumulated over kv chunks
      PE transpose -> [q, .] layout; DVE reciprocal + broadcast multiply
      assemble [128, 400] and DMA to DRAM.
  - Host scatters per-slot outputs into the final [32, 512, 400] (rows beyond
    Q_len stay zero, which implements the multiplicative q mask exactly).
"""

import math
import random

import numpy as np

import concourse.bacc as bacc
import concourse.bass as bass
import concourse.tile as tile
from concourse import mybir
from concourse.bass_utils import run_bass_kernel_spmd

B, LQ, LK = 32, 512, 512
H, D = 20, 20
OUT_DIM = H * D  # 400
N_CORES = 8
N_SLOTS = B // N_CORES  # 4
QCH = 128
KCH = 128
NG = 5  # head groups
HPG = 4  # heads per group (at partition offsets 0/32/64/96)
VW = H * 21 + 12  # 432 (even, for fp32r): per-head 20 dims + 1 ones col,
                  # padded so a 32-wide lhsT slice exists for every head
SCALE = 1.0 / math.sqrt(D)

F32 = mybir.dt.float32

# Perf knobs
USE_F32R = True  # bitcast matmul operands to float32r (fast fp32 path)
TRACE = False  # set True to capture NTFF profile (slower)
LAST_RESULT = None  # BassKernelResults of the last run (for test harness)


# ----------------------------------------------------------------- planning

def _plan(q_len, v_len):
    """Group 32 batches into N_SLOTS groups of N_CORES, minimizing baked cost.

    Returns list of (nq, nkc, batches[8]) sorted big->small."""
    nqc = [max(1, math.ceil(min(int(q), LQ) / QCH)) for q in q_len]
    kv_eff = [LK if int(v) <= 0 else min(int(v), LK) for v in v_len]
    nkc = [math.ceil(k / KCH) for k in kv_eff]
    cost = [a * b for a, b in zip(nqc, nkc)]
    order = sorted(range(B), key=lambda b: -cost[b])

    def baked(gs):
        t = 0
        for g in gs:
            if g:
                t += max(nqc[b] for b in g) * max(nkc[b] for b in g)
        return t

    groups = [[] for _ in range(N_SLOTS)]
    for b in order:
        best, bestc = None, None
        for gi in range(N_SLOTS):
            if len(groups[gi]) >= N_CORES:
                continue
            groups[gi].append(b)
            c = baked(groups)
            groups[gi].pop()
            if bestc is None or c < bestc:
                best, bestc = gi, c
        groups[best].append(b)
    rng = random.Random(0)
    cur = baked(groups)
    for _ in range(6000):
        g1, g2 = rng.randrange(N_SLOTS), rng.randrange(N_SLOTS)
        if g1 == g2:
            continue
        i1, i2 = rng.randrange(N_CORES), rng.randrange(N_CORES)
        groups[g1][i1], groups[g2][i2] = groups[g2][i2], groups[g1][i1]
        c = baked(groups)
        if c <= cur:
            cur = c
        else:
            groups[g1][i1], groups[g2][i2] = groups[g2][i2], groups[g1][i1]
    slots = []
    for g in groups:
        snq = max(nqc[b] for b in g) * QCH
        snkc = max(nkc[b] for b in g)
        slots.append((snq, snkc, list(g)))
    slots.sort(key=lambda s: -(s[0] * s[1]))
    return slots


# ------------------------------------------------------------ host packing

def _pack_qk_weights(W, bias):
    """[400, 20] linear weight -> [21, NG*128] lhsT layout (head 4g+j at
    columns 128g+32j .. +20; row 20 = bias)."""
    t = np.zeros((D + 1, NG * 128), np.float32)
    for h in range(H):
        g, j = divmod(h, HPG)
        c = g * 128 + 32 * j
        t[:D, c:c + D] = W[h * D:(h + 1) * D, :].T
        t[D, c:c + D] = bias[h * D:(h + 1) * D]
    return t


def _pack_v_weights(W, bias):
    """[400, 20] -> [21, 420] rhs layout: head h at cols 21h..21h+19,
    ones-generator col at 21h+20."""
    t = np.zeros((D + 1, VW), np.float32)
    for h in range(H):
        c = 21 * h
        t[:D, c:c + D] = W[h * D:(h + 1) * D, :].T
        t[D, c:c + D] = bias[h * D:(h + 1) * D]
        t[D, c + D] = 1.0
    return t


def _prep_qt(qs, nq):
    t = np.zeros((D + 1, nq), np.float32)
    n = min(nq, LQ)
    t[:D, :n] = qs[:n].T
    t[D, :n] = 1.0
    return t


def _prep_kvt(ks, vlen, nkv):
    """K/V sequence transposed with ones row; columns >= V_len zeroed
    (vlen==0 means "uniform -1e12 shift" in the reference == full attention)."""
    t = np.zeros((D + 1, nkv), np.float32)
    n = min(nkv, LK) if int(vlen) <= 0 else min(nkv, int(vlen))
    t[:D, :n] = ks[:n].T
    t[D, :n] = 1.0
    return t


# ------------------------------------------------------------ device build

def _emit(tc, nc, dr, slots):
    # fp32r matmul operands must come from instructions that round to fp32r;
    # DMA can't, so DMA'd tensors get one DVE rounding copy each.
    DT = mybir.dt.float32r if USE_F32R else F32
    with (
        tc.tile_pool(name="wpool", bufs=1) as wpool,
        tc.tile_pool(name="seqin", bufs=2) as seqp,
        tc.tile_pool(name="sbq", bufs=3) as sbqp,
        tc.tile_pool(name="sbk", bufs=3) as sbkp,
        tc.tile_pool(name="sbv", bufs=6) as sbvp,
        tc.tile_pool(name="sbp", bufs=4) as sbpp,
        tc.tile_pool(name="sbo", bufs=2) as sbop,
        tc.tile_pool(name="sbr", bufs=4) as sbrp,
        tc.tile_pool(name="asm", bufs=6) as asmp,
        tc.tile_pool(name="ppj", bufs=2, space="PSUM") as ppj,
        tc.tile_pool(name="pss", bufs=2, space="PSUM") as pss,
        tc.tile_pool(name="pso", bufs=1, space="PSUM") as pso,
        tc.tile_pool(name="pst", bufs=1, space="PSUM") as pst,
    ):
        def load_rounded(name, shape, pool, tag):
            raw = pool.tile(shape, F32, tag=tag + "_raw", name=name + "_raw")
            nc.sync.dma_start(raw[:], dr[name])
            if not USE_F32R:
                return raw
            t = pool.tile(shape, DT, tag=tag, name=name + "_r")
            nc.vector.tensor_copy(t[:], raw[:])
            return t

        wq = load_rounded("wq", [D + 1, NG * 128], wpool, "wq")
        wk = load_rounded("wk", [D + 1, NG * 128], wpool, "wk")
        wv = load_rounded("wv", [D + 1, VW], wpool, "wv")
        ident = load_rounded("ident", [128, 128], wpool, "ident")

        for s, (nq, nkc, _g) in enumerate(slots):
            nkv = nkc * KCH
            nqc = nq // QCH
            # 2 heads per S^T psum tile; each head's [128, nq] slice padded to a
            # full 2KB bank so no two matmul outputs share a PSUM zero region.
            hp = 2

            qt = load_rounded(f"qt{s}", [D + 1, nq], seqp, "qt")
            kt = load_rounded(f"kt{s}", [D + 1, nkv], seqp, "kt")
            vt = load_rounded(f"vt{s}", [D + 1, nkv], seqp, "vt")

            # V projection: per kv chunk -> [128, 420] (incl. masked ones cols)
            sbV = []
            for kc in range(nkc):
                pv = ppj.tile([128, 512], F32, tag="ppj")
                nc.tensor.matmul(
                    pv[:, :VW], vt[:, kc * KCH:(kc + 1) * KCH], wv[:],
                    start=True, stop=True,
                )
                v = sbvp.tile([128, VW], mybir.dt.bfloat16, tag="sbv")
                nc.vector.tensor_copy(v[:], pv[:, :VW])
                sbV.append(v)

            asms = [
                asmp.tile([128, OUT_DIM], F32, tag="asm", name=f"asm{s}_{qc}")
                for qc in range(nqc)
            ]

            for g in range(NG):
                pq = ppj.tile([128, 512], F32, tag="ppj")
                nc.tensor.matmul(
                    pq[:, :nq], wq[:, g * 128:(g + 1) * 128], qt[:],
                    start=True, stop=True,
                )
                q = sbqp.tile([128, nq], DT, tag="sbq")
                nc.vector.tensor_copy(q[:], pq[:, :nq])

                pk = ppj.tile([128, 512], F32, tag="ppj")
                nc.tensor.matmul(
                    pk[:, :nkv], wk[:, g * 128:(g + 1) * 128], kt[:],
                    start=True, stop=True,
                )
                k = sbkp.tile([128, nkv], DT, tag="sbk")
                nc.vector.tensor_copy(k[:], pk[:, :nkv])

                po = pso.tile([128, nq], F32, tag="pso")

                for kc in range(nkc):
                    # all 4 S^T matmuls back-to-back (distinct row groups ->
                    # they pipeline/overlap in the PE's 32x32 subarrays),
                    # then the exps, then the 4 O^T matmuls (distinct col
                    # groups).  Interleaving full-row-span work between
                    # row-tiled matmuls would serialize the subarrays.
                    packs = []
                    for jp in range(0, HPG, hp):
                        ps = pss.tile([128, hp, 512], F32, tag="pss",
                                      name=f"ps{s}_{g}_{kc}_{jp}")
                        for j in range(jp, jp + hp):
                            nc.tensor.matmul(
                                ps[:, j - jp, :nq],
                                k[32 * j:32 * j + D, kc * KCH:(kc + 1) * KCH],
                                q[32 * j:32 * j + D, :],
                                start=True, stop=True,
                                tile_position=(32 * j, 0),
                            )
                        packs.append(ps)
                    ptiles = []
                    for jp, ps in zip(range(0, HPG, hp), packs):
                        p = sbpp.tile([128, hp, 512], mybir.dt.bfloat16,
                                      tag="sbp", name=f"p{s}_{g}_{kc}_{jp}")
                        nc.scalar.activation(
                            p[:, :, :nq], ps[:, :, :nq],
                            mybir.ActivationFunctionType.Exp,
                            bias=0.0, scale=SCALE,
                        )
                        ptiles.append(p)
                    for jp, p in zip(range(0, HPG, hp), ptiles):
                        for j in range(jp, jp + hp):
                            h = HPG * g + j
                            # col-tiled accumulation chains touch disjoint
                            # partition ranges (32j..32j+20) of one bank; the
                            # sim's zero-region check is bank-granular, so
                            # bypass it.
                            nc.tensor.matmul(
                                po[32 * j:32 * j + 32, :],
                                sbV[kc][:, 21 * h:21 * h + 32],
                                p[:, j - jp, :nq],
                                start=(kc == 0), stop=(kc == nkc - 1),
                                tile_position=(0, 32 * j),
                                skip_group_check=True,
                            )

                o = sbop.tile([128, nq], DT, tag="sbo")
                nc.vector.tensor_copy(o[:], po[:])
                for qc in range(nqc):
                    pt = pst.tile([128, 128], DT, tag="pst")
                    nc.tensor.transpose(pt[:], o[:, qc * QCH:(qc + 1) * QCH], ident[:])
                    # f32r bits are valid f32; read back as f32 for DVE ops
                    ptb = pt.bitcast(F32).rearrange("p (j c) -> p j c", j=HPG)
                    r = sbrp.tile([128, HPG], F32, tag="sbr")
                    nc.vector.reciprocal(r[:], ptb[:, :, D])
                    nc.vector.tensor_mul(
                        asms[qc][:, g * 80:(g + 1) * 80]
                            .rearrange("p (j d) -> p j d", j=HPG),
                        ptb[:, :, 0:D],
                        r.unsqueeze(2).broadcast_to([128, HPG, D]),
                    )

            for qc in range(nqc):
                nc.sync.dma_start(
                    dr[f"o{s}"][qc * QCH:(qc + 1) * QCH, :], asms[qc][:]
                )


def _build_nc(slots):
    nc = bacc.Bacc(
        "TRN2",
        target_bir_lowering=False,
        debug=False,
        enable_asserts=False,
        num_devices=N_CORES,
    )
    dr = {}
    for s, (nq, nkc, _grp) in enumerate(slots):
        nkv = nkc * KCH
        dr[f"qt{s}"] = nc.dram_tensor(f"qt{s}", [D + 1, nq], F32, kind="ExternalInput").ap()
        dr[f"kt{s}"] = nc.dram_tensor(f"kt{s}", [D + 1, nkv], F32, kind="ExternalInput").ap()
        dr[f"vt{s}"] = nc.dram_tensor(f"vt{s}", [D + 1, nkv], F32, kind="ExternalInput").ap()
        dr[f"o{s}"] = nc.dram_tensor(f"o{s}", [nq, OUT_DIM], F32, kind="ExternalOutput").ap()
    dr["wq"] = nc.dram_tensor("wq", [D + 1, NG * 128], F32, kind="ExternalInput").ap()
    dr["wk"] = nc.dram_tensor("wk", [D + 1, NG * 128], F32, kind="ExternalInput").ap()
    dr["wv"] = nc.dram_tensor("wv", [D + 1, VW], F32, kind="ExternalInput").ap()
    dr["ident"] = nc.dram_tensor("ident", [128, 128], F32, kind="ExternalInput").ap()

    with tile.TileContext(nc) as tc:
        _emit(tc, nc, dr, slots)
    nc.compile()
    return nc


# ----------------------------------------------------------------- driver

def kernel(**inputs):
    global LAST_RESULT
    Q_seq = np.ascontiguousarray(np.asarray(inputs["Q_seq"], dtype=np.float32))
    K_seq = np.ascontiguousarray(np.asarray(inputs["K_seq"], dtype=np.float32))
    V_seq = np.ascontiguousarray(np.asarray(inputs["V_seq"], dtype=np.float32))
    Q_len = np.asarray(inputs["Q_len"]).reshape(-1).astype(np.int64)
    V_len = np.asarray(inputs["V_len"]).reshape(-1).astype(np.int64)
    WQ_w = np.asarray(inputs["WQ_w"], dtype=np.float32)
    WQ_b = np.asarray(inputs["WQ_b"], dtype=np.float32)
    WK_w = np.asarray(inputs["WK_w"], dtype=np.float32)
    WK_b = np.asarray(inputs["WK_b"], dtype=np.float32)
    WV_w = np.asarray(inputs["WV_w"], dtype=np.float32)
    WV_b = np.asarray(inputs["WV_b"], dtype=np.float32)

    slots = _plan(Q_len, V_len)
    nc = _build_nc(slots)

    wq = _pack_qk_weights(WQ_w, WQ_b)
    wk = _pack_qk_weights(WK_w, WK_b)
    wv = _pack_v_weights(WV_w, WV_b)
    ident = np.eye(128, dtype=np.float32)

    in_maps = []
    for c in range(N_CORES):
        m = {"wq": wq, "wk": wk, "wv": wv, "ident": ident}
        for s, (nq, nkc, grp) in enumerate(slots):
            b = grp[c]
            nkv = nkc * KCH
            m[f"qt{s}"] = _prep_qt(Q_seq[b], nq)
            m[f"kt{s}"] = _prep_kvt(K_seq[b], V_len[b], nkv)
            m[f"vt{s}"] = _prep_kvt(V_seq[b], V_len[b], nkv)
        in_maps.append(m)

    res = run_bass_kernel_spmd(
        nc, in_maps, core_ids=list(range(N_CORES)), trace=TRACE
    )
    LAST_RESULT = res

    out = np.zeros((B, LQ, OUT_DIM), np.float32)
    for c in range(N_CORES):
        for s, (_nq, _nkc, grp) in enumerate(slots):
            b = grp[c]
            ql = int(Q_len[b])
            if ql > 0:
                out[b, :ql] = res.results[c][f"o{s}"][:ql]
    return out
